# revision 37
# baseline (speedup 1.0000x reference)
"""Trainium2 Bass kernel for nn_BigramModel (unigram/bigram/trigram interpolated LM).

Sharding (per hint): replicate all tables, shard text [256, 64] along batch
across 8 cores -> [256, 8] per core; every gather is core-local.

Numerics: all tables and the output travel as bf16 (harness tolerance is
2e-2; measured rel err 4.5e-3), halving HBM traffic vs f32 to ~34 MB/core
(16.8 MB bigram-row reads + 16.8 MB log-prob writes + index minis).

Host prep folds the elementwise pipeline into the tables:
  probs ~ q = (bigram + 0.75*uni)[cur] + 0.75*tri[ridx]   (scale 1/ALPHA
  cancels in the normalization), so the device table is
  aug [V, 4098] bf16 = bigram + 0.75*uni with the row sum in column V --
  the kernel never runs a V-wide unigram add or row reduction.
  tri_rows are pre-scaled by 0.75.

Per 128-token tile (16/core), software-pipelined:
  - prep (LOOKAHEAD=3 tiles ahead): gather trigram row ids from tri_map
    (indirect DMA on the flat key prev*4096+cur), derive the bounds-check
    index (miss -> 65535 > K-1 so the row gather OOB-skips; a skipped row
    costs a 4-byte null packet, no HBM read), the {0,1} hit mask, and
    zero the trigram landing tile (via a u32-bitcast view: half the DVE
    elements for the same bytes).
  - bigram gathers (128 augmented rows, 8.2KB each) run 2 tiles AHEAD of
    the trigram gathers, and the first two are issued before any prep so
    the bulk read stream starts as soon as the token ids land.
  - compute, skewed ONE iteration behind the gathers so vector/scalar ops
    never head-of-line-wait on a just-issued DMA: z = rowsum + 0.75*hit,
    r = 1/z ([128,1] minis), q = tri + bi (one 2x-mode bf16 add),
    out_row = Ln(r*q + EPS) on the scalar engine, DMA out in bf16.
Seq positions 0,1 never take the trigram branch; their keys are remapped to
a known tri_map miss at trace time (k_miss) instead of patching per tile.

Why this shape: gpsimd SWDGE descriptor emission (~1.5us per 128-row
indirect op, 48 ops) and HBM bytes are the scarce resources; the vector
engine only does one cheap pass per tile and the Ln runs on the otherwise
idle scalar engine.  Measured 127.9us/run on HW (f32 baseline: 309.5us),
DMA active ~101us on ~34MB -> ~330GB/s, at the per-core HBM roofline.
"""

import numpy as np
import ml_dtypes

import concourse.bass as bass
import concourse.bacc as bacc
import concourse.tile as tile
from concourse import mybir
from concourse.bass_utils import run_bass_kernel_spmd

V = 4096
S = 256
B = 64
K = 20000
NCORES = 8
BS = B // NCORES  # 8 batch columns per core
P = 128
VA = V + 2  # augmented row: [0:V] = bigram + 0.75*uni, [V] = row sum, [V+1] = pad

ALPHA = 0.4
BETA = 0.3
C1 = 1.0 - ALPHA - BETA  # 0.3
R_UNI = C1 / ALPHA  # 0.75
R_TRI = BETA / ALPHA  # 0.75
EPS = 1e-10

f32 = mybir.dt.float32
bf16 = mybir.dt.bfloat16
i32 = mybir.dt.int32
BF16 = ml_dtypes.bfloat16


def build_nc(n_b: int = BS, k_miss: int | None = None) -> bass.Bass:
    nc = bacc.Bacc("TRN2", num_devices=NCORES)

    text = nc.dram_tensor("text", [S, n_b], i32, kind="ExternalInput")
    bigram = nc.dram_tensor("bigram_table", [V, VA], bf16, kind="ExternalInput")
    tri_rows = nc.dram_tensor("tri_rows", [K, V], bf16, kind="ExternalInput")
    tri_map = nc.dram_tensor("tri_map", [V * V, 1], i32, kind="ExternalInput")
    out = nc.dram_tensor("out", [S, n_b * V], bf16, kind="ExternalOutput")

    n_h = S // P  # seq halves (2)
    n_tiles = n_b * n_h
    LOOKAHEAD = 3
    GBUFS = LOOKAHEAD + 4

    with tile.TileContext(nc) as tc:
        with (
            tc.tile_pool(name="const", bufs=1) as const_pool,
            tc.tile_pool(name="half", bufs=n_h) as half,
            tc.tile_pool(name="bi", bufs=GBUFS) as bi_pool,
            tc.tile_pool(name="tri", bufs=GBUFS) as tri_pool,
            tc.tile_pool(name="ot", bufs=4) as out_pool,
            tc.tile_pool(name="small", bufs=n_tiles) as small,
        ):
            eps_b = const_pool.tile([P, 1], f32, tag="eps_b")
            nc.vector.memset(eps_b[:], EPS)

            # ---- phase 1: per-half (128 x n_b) token index prep ----
            curs, fks = [], []
            for h in range(n_h):
                s0 = h * P
                cur = half.tile([P, n_b], i32, tag="cur")
                nc.sync.dma_start(cur[:], text[s0 : s0 + P, :])
                prv = half.tile([P, n_b], i32, tag="prv")
                if h == 0:
                    nc.sync.dma_start(prv[0:1, :], text[0:1, :])
                    nc.sync.dma_start(prv[1:P, :], text[0 : P - 1, :])
                else:
                    nc.sync.dma_start(prv[:], text[s0 - 1 : s0 + P - 1, :])

                # flat trigram key = prev * 4096 + cur (exact, < 2^24)
                fk = half.tile([P, n_b], i32, tag="fk")
                nc.vector.scalar_tensor_tensor(
                    out=fk[:],
                    in0=prv[:],
                    scalar=V,
                    in1=cur[:],
                    op0=mybir.AluOpType.mult,
                    op1=mybir.AluOpType.add,
                )
                if h == 0 and k_miss is not None:
                    # seq positions 0,1 never take the trigram branch: remap
                    # their keys to a known tri_map miss
                    nc.vector.memset(fk[0:2, :], k_miss)
                curs.append(cur)
                fks.append(fk)

            # ---- phase 2: software-pipelined per-tile work ----
            # gpsimd (SWDGE Q7 descriptor emission + ring backpressure) and
            # HBM bytes are the scarce resources: keep the gpsimd queue free
            # of data-dependent stalls by running index prep LOOKAHEAD tiles
            # ahead, and keep the combine on the vector engine in cheap 2x
            # ops (tri tile zeroed, plain tensor add).
            tiles = [(b, h) for b in range(n_b) for h in range(n_h)]
            risks, hits, tris = {}, {}, {}

            def issue_prep(t):
                b, h = tiles[t]
                ridx = small.tile([P, 1], i32, tag="ridx")
                nc.gpsimd.indirect_dma_start(
                    out=ridx[:],
                    out_offset=None,
                    in_=tri_map[:],
                    in_offset=bass.IndirectOffsetOnAxis(
                        ap=fks[h][:, b : b + 1], axis=0
                    ),
                )
                if h == 0 and k_miss is None:
                    nc.vector.memset(ridx[0:2, :], -1)
                # zero the tri landing tile first: it is independent of the
                # ridx gather, so it absorbs the gather's drain latency
                # before risk/hit read ridx; u32 view halves the DVE
                # element count (same bytes)
                tri = tri_pool.tile([P, V], bf16, tag="tri")
                nc.vector.memset(tri[:].bitcast(mybir.dt.uint32), 0)
                # miss (-1) -> 65535 which fails bounds_check -> skipped
                risk = small.tile([P, 1], i32, tag="risk")
                nc.vector.tensor_scalar(
                    out=risk[:],
                    in0=ridx[:],
                    scalar1=0xFFFF,
                    scalar2=None,
                    op0=mybir.AluOpType.bitwise_and,
                )
                # hit indicator in {0.0, 1.0}
                hit = small.tile([P, 1], f32, tag="hit")
                nc.vector.tensor_scalar(
                    out=hit[:],
                    in0=ridx[:],
                    scalar1=0,
                    scalar2=None,
                    op0=mybir.AluOpType.is_ge,
                )
                risks[t] = risk
                hits[t] = hit
                tris[t] = tri

            bis = {}

            def issue_bi(t):
                b, h = tiles[t]
                bi = bi_pool.tile([P, VA], bf16, tag="bi")
                nc.gpsimd.indirect_dma_start(
                    out=bi[:],
                    out_offset=None,
                    in_=bigram[:],
                    in_offset=bass.IndirectOffsetOnAxis(
                        ap=curs[h][:, b : b + 1], axis=0
                    ),
                )
                bis[t] = bi

            def issue_tri(t):
                nc.gpsimd.indirect_dma_start(
                    out=tris[t][:],
                    out_offset=None,
                    in_=tri_rows[:],
                    in_offset=bass.IndirectOffsetOnAxis(
                        ap=risks[t][:, :1], axis=0
                    ),
                    bounds_check=K - 1,
                    oob_is_err=False,
                )

            def issue_compute(t):
                b, h = tiles[t]
                s0 = h * P
                bi, hit, tri = bis[t], hits[t], tris[t]

                # z = rowsum + 0.75*hit (sum col; EPS/ALPHA = 2.5e-10 is
                # below f32/bf16 resolution of z ~ 1.75 so reference's +EPS
                # in the denominator is a no-op here)
                z = small.tile([P, 1], f32, tag="z")
                nc.vector.scalar_tensor_tensor(
                    out=z[:],
                    in0=hit[:, :1],
                    scalar=R_TRI,
                    in1=bi[:, V : V + 1],
                    op0=mybir.AluOpType.mult,
                    op1=mybir.AluOpType.add,
                )
                r = small.tile([P, 1], f32, tag="r")
                nc.vector.reciprocal(r[:], z[:])

                # q = tri + bi (tri pre-scaled by 0.75, zero on miss; 2x TT)
                nc.vector.tensor_tensor(
                    out=bi[:, 0:V],
                    in0=tri[:],
                    in1=bi[:, 0:V],
                    op=mybir.AluOpType.add,
                )

                ot = out_pool.tile([P, V], bf16, tag="ot")
                nc.scalar.activation(
                    out=ot[:],
                    in_=bi[:, 0:V],
                    func=mybir.ActivationFunctionType.Ln,
                    bias=eps_b[:, :1],
                    scale=r[:, :1],
                )

                nc.sync.dma_start(out[s0 : s0 + P, b * V : (b + 1) * V], ot[:])

            # start the bulk read stream immediately (bigram gathers only
            # need the token ids), then index prep; trigram gathers trail
            # the bigram gathers by 2 tiles, so by the time compute(t) runs
            # its bigram tile drained long ago and its only fresh dependency
            # is the trigram gather (mostly 4-byte null packets, ~1us)
            for t in range(min(2, n_tiles)):
                issue_bi(t)
            for t in range(min(LOOKAHEAD, n_tiles)):
                issue_prep(t)
            for t in range(n_tiles):
                if t + 2 < n_tiles:
                    issue_bi(t + 2)
                issue_tri(t)
                issue_compute(t)
                if t + LOOKAHEAD < n_tiles:
                    issue_prep(t + LOOKAHEAD)

    nc.finalize()
    return nc


def _prep_inputs(text, unigram, bigram_table, tri_rows, tri_map):
    """Shared (replicated) device arrays, keyed by dram tensor name."""
    text = np.ascontiguousarray(np.asarray(text, dtype=np.int32))
    uni = np.asarray(unigram, np.float32).reshape(1, V)
    bt = np.asarray(bigram_table, np.float32) + R_UNI * uni  # fold unigram in
    aug = np.zeros((V, VA), dtype=BF16)
    aug[:, :V] = bt.astype(BF16)
    aug[:, V] = bt.sum(axis=1).astype(BF16)
    tr = np.ascontiguousarray(
        (np.asarray(tri_rows, np.float32) * R_TRI).astype(BF16)
    )
    tm = np.ascontiguousarray(np.asarray(tri_map, np.int32).reshape(V * V, 1))
    shared = {
        "text": text,
        "bigram_table": np.ascontiguousarray(aug),
        "tri_rows": np.ascontiguousarray(tr),
        "tri_map": tm,
    }
    # any key absent from the trigram dict (used to force seq pos 0,1 to miss)
    k_miss = int(np.flatnonzero(tm[:, 0] < 0)[0])
    return shared, k_miss


def make_in_maps(shared):
    text = shared["text"]
    in_maps = []
    for c in range(NCORES):
        m = dict(shared)
        m["text"] = np.ascontiguousarray(text[:, c * BS : (c + 1) * BS])
        in_maps.append(m)
    return in_maps


def kernel(text, unigram, bigram_table, tri_rows, tri_map, _trace=False, _trace_kwargs=None):
    shared, k_miss = _prep_inputs(text, unigram, bigram_table, tri_rows, tri_map)
    nc = build_nc(BS, k_miss=k_miss)
    in_maps = make_in_maps(shared)
    res = run_bass_kernel_spmd(
        nc,
        in_maps,
        core_ids=list(range(NCORES)),
        trace=_trace,
        **(_trace_kwargs or {}),
    )
    outs = [
        np.asarray(res.results[c]["out"]).astype(np.float32).reshape(S, BS, V)
        for c in range(NCORES)
    ]
    full = np.concatenate(outs, axis=1)
    if _trace:
        return full, res
    return full


# revision 39
# speedup vs baseline: 1.0158x; 1.0158x over previous
"""Trainium2 Bass kernel for nn_BigramModel (unigram/bigram/trigram interpolated LM).

Sharding (per hint): replicate all tables, shard text [256, 64] along batch
across 8 cores -> [256, 8] per core; every gather is core-local.

Numerics: all tables and the output travel as bf16 (harness tolerance is
2e-2; measured rel err 4.5e-3), halving HBM traffic vs f32 to ~34 MB/core
(16.8 MB bigram-row reads + 16.8 MB log-prob writes + index minis).

Host prep folds the elementwise pipeline into the tables:
  probs ~ q = (bigram + 0.75*uni)[cur] + 0.75*tri[ridx]   (scale 1/ALPHA
  cancels in the normalization), so the device table is
  aug [V, 4098] bf16 = bigram + 0.75*uni with the row sum in column V --
  the kernel never runs a V-wide unigram add or row reduction.
  tri_rows are pre-scaled by 0.75.

Per 128-token tile (16/core), software-pipelined:
  - prep (LOOKAHEAD=3 tiles ahead): gather trigram row ids from tri_map
    (indirect DMA on the flat key prev*4096+cur), derive the bounds-check
    index (miss -> 65535 > K-1 so the row gather OOB-skips; a skipped row
    costs a 4-byte null packet, no HBM read), the {0,1} hit mask, and
    zero the trigram landing tile.
  - gathers: 128 augmented bigram rows (8.2KB each) + trigram rows.
  - compute, skewed ONE iteration behind the gathers so vector/scalar ops
    never head-of-line-wait on a just-issued DMA: z = rowsum + 0.75*hit,
    r = 1/z ([128,1] minis), q = tri + bi (one 2x-mode bf16 add),
    out_row = Ln(r*q + EPS) on the scalar engine, DMA out in bf16.
Seq positions 0,1 never take the trigram branch; their keys are remapped to
a known tri_map miss at trace time (k_miss) instead of patching per tile.

Why this shape: gpsimd SWDGE descriptor emission (~1.5us per 128-row
indirect op, 48 ops) and HBM bytes are the scarce resources; the vector
engine only does one cheap pass per tile and the Ln runs on the otherwise
idle scalar engine.  Measured 133us/run on HW (f32 baseline: 309.5us),
with DMA active ~104us on ~34MB -> at the per-core HBM roofline.
"""

import numpy as np
import ml_dtypes

import concourse.bass as bass
import concourse.bacc as bacc
import concourse.tile as tile
from concourse import mybir
from concourse.bass_utils import run_bass_kernel_spmd

V = 4096
S = 256
B = 64
K = 20000
NCORES = 8
BS = B // NCORES  # 8 batch columns per core
P = 128
VA = V + 2  # augmented row: [0:V] = bigram + 0.75*uni, [V] = row sum, [V+1] = pad

ALPHA = 0.4
BETA = 0.3
C1 = 1.0 - ALPHA - BETA  # 0.3
R_UNI = C1 / ALPHA  # 0.75
R_TRI = BETA / ALPHA  # 0.75
EPS = 1e-10

f32 = mybir.dt.float32
bf16 = mybir.dt.bfloat16
i32 = mybir.dt.int32
BF16 = ml_dtypes.bfloat16


def build_nc(n_b: int = BS, k_miss: int | None = None) -> bass.Bass:
    nc = bacc.Bacc("TRN2", num_devices=NCORES)

    text = nc.dram_tensor("text", [S, n_b], i32, kind="ExternalInput")
    bigram = nc.dram_tensor("bigram_table", [V, VA], bf16, kind="ExternalInput")
    tri_rows = nc.dram_tensor("tri_rows", [K, V], bf16, kind="ExternalInput")
    tri_map = nc.dram_tensor("tri_map", [V * V, 1], i32, kind="ExternalInput")
    out = nc.dram_tensor("out", [S, n_b * V], bf16, kind="ExternalOutput")

    n_h = S // P  # seq halves (2)
    n_tiles = n_b * n_h
    LOOKAHEAD = 3
    GBUFS = LOOKAHEAD + 4

    with tile.TileContext(nc) as tc:
        with (
            tc.tile_pool(name="const", bufs=1) as const_pool,
            tc.tile_pool(name="half", bufs=n_h) as half,
            tc.tile_pool(name="bi", bufs=GBUFS) as bi_pool,
            tc.tile_pool(name="tri", bufs=GBUFS) as tri_pool,
            tc.tile_pool(name="ot", bufs=4) as out_pool,
            tc.tile_pool(name="small", bufs=n_tiles) as small,
        ):
            eps_b = const_pool.tile([P, 1], f32, tag="eps_b")
            nc.vector.memset(eps_b[:], EPS)
            # scalar-engine warm-up: absorb the Ln table load and first-
            # dispatch overhead during the lead-in so tile 0's real
            # activation isn't taxed (~10us otherwise)
            warm = const_pool.tile([P, 1], f32, tag="warm")
            nc.scalar.activation(
                out=warm[:],
                in_=eps_b[:],
                func=mybir.ActivationFunctionType.Ln,
                bias=1.0,
                scale=1.0,
            )

            # ---- phase 1: per-half (128 x n_b) token index prep ----
            curs, fks = [], []
            for h in range(n_h):
                s0 = h * P
                cur = half.tile([P, n_b], i32, tag="cur")
                nc.sync.dma_start(cur[:], text[s0 : s0 + P, :])
                prv = half.tile([P, n_b], i32, tag="prv")
                if h == 0:
                    nc.sync.dma_start(prv[0:1, :], text[0:1, :])
                    nc.sync.dma_start(prv[1:P, :], text[0 : P - 1, :])
                else:
                    nc.sync.dma_start(prv[:], text[s0 - 1 : s0 + P - 1, :])

                # flat trigram key = prev * 4096 + cur (exact, < 2^24)
                fk = half.tile([P, n_b], i32, tag="fk")
                nc.vector.scalar_tensor_tensor(
                    out=fk[:],
                    in0=prv[:],
                    scalar=V,
                    in1=cur[:],
                    op0=mybir.AluOpType.mult,
                    op1=mybir.AluOpType.add,
                )
                if h == 0 and k_miss is not None:
                    # seq positions 0,1 never take the trigram branch: remap
                    # their keys to a known tri_map miss
                    nc.vector.memset(fk[0:2, :], k_miss)
                curs.append(cur)
                fks.append(fk)

            # ---- phase 2: software-pipelined per-tile work ----
            # gpsimd (SWDGE Q7 descriptor emission + ring backpressure) and
            # HBM bytes are the scarce resources: keep the gpsimd queue free
            # of data-dependent stalls by running index prep LOOKAHEAD tiles
            # ahead, and keep the combine on the vector engine in cheap 2x
            # ops (tri tile zeroed, plain tensor add).
            tiles = [(b, h) for b in range(n_b) for h in range(n_h)]
            risks, hits, tris = {}, {}, {}

            def issue_prep(t):
                b, h = tiles[t]
                ridx = small.tile([P, 1], i32, tag="ridx")
                nc.gpsimd.indirect_dma_start(
                    out=ridx[:],
                    out_offset=None,
                    in_=tri_map[:],
                    in_offset=bass.IndirectOffsetOnAxis(
                        ap=fks[h][:, b : b + 1], axis=0
                    ),
                )
                if h == 0 and k_miss is None:
                    nc.vector.memset(ridx[0:2, :], -1)
                # miss (-1) -> 65535 which fails bounds_check -> skipped
                risk = small.tile([P, 1], i32, tag="risk")
                nc.vector.tensor_scalar(
                    out=risk[:],
                    in0=ridx[:],
                    scalar1=0xFFFF,
                    scalar2=None,
                    op0=mybir.AluOpType.bitwise_and,
                )
                # hit indicator in {0.0, 1.0}
                hit = small.tile([P, 1], f32, tag="hit")
                nc.vector.tensor_scalar(
                    out=hit[:],
                    in0=ridx[:],
                    scalar1=0,
                    scalar2=None,
                    op0=mybir.AluOpType.is_ge,
                )
                # zero the tri tile ahead of time so skipped (miss) rows
                # contribute exactly 0 to the add; u32 view halves the DVE
                # element count (same bytes)
                tri = tri_pool.tile([P, V], bf16, tag="tri")
                nc.vector.memset(tri[:].bitcast(mybir.dt.uint32), 0)
                risks[t] = risk
                hits[t] = hit
                tris[t] = tri

            bis = {}

            def issue_bi(t):
                b, h = tiles[t]
                bi = bi_pool.tile([P, VA], bf16, tag="bi")
                nc.gpsimd.indirect_dma_start(
                    out=bi[:],
                    out_offset=None,
                    in_=bigram[:],
                    in_offset=bass.IndirectOffsetOnAxis(
                        ap=curs[h][:, b : b + 1], axis=0
                    ),
                )
                bis[t] = bi

            def issue_tri(t):
                nc.gpsimd.indirect_dma_start(
                    out=tris[t][:],
                    out_offset=None,
                    in_=tri_rows[:],
                    in_offset=bass.IndirectOffsetOnAxis(
                        ap=risks[t][:, :1], axis=0
                    ),
                    bounds_check=K - 1,
                    oob_is_err=False,
                )

            def issue_compute(t):
                b, h = tiles[t]
                s0 = h * P
                bi, hit, tri = bis[t], hits[t], tris[t]

                # z = rowsum + 0.75*hit (sum col; EPS/ALPHA = 2.5e-10 is
                # below f32/bf16 resolution of z ~ 1.75 so reference's +EPS
                # in the denominator is a no-op here)
                z = small.tile([P, 1], f32, tag="z")
                nc.vector.scalar_tensor_tensor(
                    out=z[:],
                    in0=hit[:, :1],
                    scalar=R_TRI,
                    in1=bi[:, V : V + 1],
                    op0=mybir.AluOpType.mult,
                    op1=mybir.AluOpType.add,
                )
                r = small.tile([P, 1], f32, tag="r")
                nc.vector.reciprocal(r[:], z[:])

                # q = tri + bi (tri pre-scaled by 0.75, zero on miss; 2x TT)
                nc.vector.tensor_tensor(
                    out=bi[:, 0:V],
                    in0=tri[:],
                    in1=bi[:, 0:V],
                    op=mybir.AluOpType.add,
                )

                ot = out_pool.tile([P, V], bf16, tag="ot")
                nc.scalar.activation(
                    out=ot[:],
                    in_=bi[:, 0:V],
                    func=mybir.ActivationFunctionType.Ln,
                    bias=eps_b[:, :1],
                    scale=r[:, :1],
                )

                nc.sync.dma_start(out[s0 : s0 + P, b * V : (b + 1) * V], ot[:])

            # start the bulk read stream immediately (bigram gathers only
            # need the token ids), then index prep; trigram gathers trail
            # the bigram gathers by 2 tiles and compute trails by one more,
            # so no engine ever head-of-line-waits on a just-issued DMA
            for t in range(min(2, n_tiles)):
                issue_bi(t)
            for t in range(min(LOOKAHEAD, n_tiles)):
                issue_prep(t)
            for t in range(n_tiles + 1):
                if t + 2 < n_tiles:
                    issue_bi(t + 2)
                if t < n_tiles:
                    issue_tri(t)
                if t >= 1:
                    issue_compute(t - 1)
                if t + LOOKAHEAD < n_tiles:
                    issue_prep(t + LOOKAHEAD)

    nc.finalize()
    return nc


def _prep_inputs(text, unigram, bigram_table, tri_rows, tri_map):
    """Shared (replicated) device arrays, keyed by dram tensor name."""
    text = np.ascontiguousarray(np.asarray(text, dtype=np.int32))
    uni = np.asarray(unigram, np.float32).reshape(1, V)
    bt = np.asarray(bigram_table, np.float32) + R_UNI * uni  # fold unigram in
    aug = np.zeros((V, VA), dtype=BF16)
    aug[:, :V] = bt.astype(BF16)
    aug[:, V] = bt.sum(axis=1).astype(BF16)
    tr = np.ascontiguousarray(
        (np.asarray(tri_rows, np.float32) * R_TRI).astype(BF16)
    )
    tm = np.ascontiguousarray(np.asarray(tri_map, np.int32).reshape(V * V, 1))
    shared = {
        "text": text,
        "bigram_table": np.ascontiguousarray(aug),
        "tri_rows": np.ascontiguousarray(tr),
        "tri_map": tm,
    }
    # any key absent from the trigram dict (used to force seq pos 0,1 to miss)
    k_miss = int(np.flatnonzero(tm[:, 0] < 0)[0])
    return shared, k_miss


def make_in_maps(shared):
    text = shared["text"]
    in_maps = []
    for c in range(NCORES):
        m = dict(shared)
        m["text"] = np.ascontiguousarray(text[:, c * BS : (c + 1) * BS])
        in_maps.append(m)
    return in_maps


def kernel(text, unigram, bigram_table, tri_rows, tri_map, _trace=False, _trace_kwargs=None):
    shared, k_miss = _prep_inputs(text, unigram, bigram_table, tri_rows, tri_map)
    nc = build_nc(BS, k_miss=k_miss)
    in_maps = make_in_maps(shared)
    res = run_bass_kernel_spmd(
        nc,
        in_maps,
        core_ids=list(range(NCORES)),
        trace=_trace,
        **(_trace_kwargs or {}),
    )
    outs = [
        np.asarray(res.results[c]["out"]).astype(np.float32).reshape(S, BS, V)
        for c in range(NCORES)
    ]
    full = np.concatenate(outs, axis=1)
    if _trace:
        return full, res
    return full


# revision 44
# speedup vs baseline: 1.0238x; 1.0079x over previous
"""Trainium2 Bass kernel for nn_BigramModel (unigram/bigram/trigram interpolated LM).

Sharding (per hint): replicate all tables, shard text [256, 64] along batch
across 8 cores -> [256, 8] per core; every gather is core-local.

Numerics: all tables and the output travel as bf16 (harness tolerance is
2e-2; measured rel err 4.5e-3), halving HBM traffic vs f32 to ~34 MB/core
(16.8 MB bigram-row reads + 16.8 MB log-prob writes + index minis).

Host prep folds the elementwise pipeline into the tables:
  probs ~ q = (bigram + 0.75*uni)[cur] + 0.75*tri[ridx]   (scale 1/ALPHA
  cancels in the normalization), so the device table is
  aug [V, 4098] bf16 = bigram + 0.75*uni with the row sum in column V --
  the kernel never runs a V-wide unigram add or row reduction.
  tri_rows are pre-scaled by 0.75.

Per 128-token tile (16/core), software-pipelined:
  - prep (LOOKAHEAD=3 tiles ahead): gather trigram row ids from tri_map
    (indirect DMA on the flat key prev*4096+cur), derive the bounds-check
    index (miss -> 65535 > K-1 so the row gather OOB-skips; a skipped row
    costs a 4-byte null packet, no HBM read), the {0,1} hit mask, and
    zero the trigram landing tile (via a u32-bitcast view: half the DVE
    elements for the same bytes).
  - bigram gathers (128 augmented rows, 8.2KB each) run 2 tiles AHEAD of
    the trigram gathers, and the first two are issued before any prep so
    the bulk read stream starts as soon as the token ids land.
  - compute, skewed ONE iteration behind the gathers so vector/scalar ops
    never head-of-line-wait on a just-issued DMA: z = rowsum + 0.75*hit,
    r = 1/z ([128,1] minis), q = tri + bi (one 2x-mode bf16 add),
    out_row = Ln(r*q + EPS) on the scalar engine, DMA out in bf16.
Seq positions 0,1 never take the trigram branch; their keys are remapped to
a known tri_map miss at trace time (k_miss) instead of patching per tile.

Why this shape: gpsimd SWDGE descriptor emission (~1.5us per 128-row
indirect op, 48 ops) and HBM bytes are the scarce resources; the vector
engine only does one cheap pass per tile and the Ln runs on the otherwise
idle scalar engine.  Measured 127.9us/run on HW (f32 baseline: 309.5us),
DMA active ~101us on ~34MB -> ~330GB/s, at the per-core HBM roofline.
"""

import numpy as np
import ml_dtypes

import concourse.bass as bass
import concourse.bacc as bacc
import concourse.tile as tile
from concourse import mybir
from concourse.bass_utils import run_bass_kernel_spmd

V = 4096
S = 256
B = 64
K = 20000
NCORES = 8
BS = B // NCORES  # 8 batch columns per core
P = 128
VA = V + 2  # augmented row: [0:V] = bigram + 0.75*uni, [V] = row sum, [V+1] = pad

ALPHA = 0.4
BETA = 0.3
C1 = 1.0 - ALPHA - BETA  # 0.3
R_UNI = C1 / ALPHA  # 0.75
R_TRI = BETA / ALPHA  # 0.75
EPS = 1e-10

f32 = mybir.dt.float32
bf16 = mybir.dt.bfloat16
i32 = mybir.dt.int32
BF16 = ml_dtypes.bfloat16


def build_nc(n_b: int = BS, k_miss: int | None = None) -> bass.Bass:
    nc = bacc.Bacc("TRN2", num_devices=NCORES)

    text = nc.dram_tensor("text", [S, n_b], i32, kind="ExternalInput")
    bigram = nc.dram_tensor("bigram_table", [V, VA], bf16, kind="ExternalInput")
    tri_rows = nc.dram_tensor("tri_rows", [K, V], bf16, kind="ExternalInput")
    tri_map = nc.dram_tensor("tri_map", [V * V, 1], i32, kind="ExternalInput")
    out = nc.dram_tensor("out", [S, n_b * V], bf16, kind="ExternalOutput")

    n_h = S // P  # seq halves (2)
    n_tiles = n_b * n_h
    LOOKAHEAD = 3
    GBUFS = LOOKAHEAD + 5

    with tile.TileContext(nc) as tc:
        with (
            tc.tile_pool(name="const", bufs=1) as const_pool,
            tc.tile_pool(name="half", bufs=n_h) as half,
            tc.tile_pool(name="bi", bufs=GBUFS) as bi_pool,
            tc.tile_pool(name="tri", bufs=GBUFS) as tri_pool,
            tc.tile_pool(name="ot", bufs=6) as out_pool,
            tc.tile_pool(name="small", bufs=n_tiles) as small,
        ):
            eps_b = const_pool.tile([P, 1], f32, tag="eps_b")
            nc.vector.memset(eps_b[:], EPS)

            # ---- phase 1: per-half (128 x n_b) token index prep ----
            curs, fks = [], []
            for h in range(n_h):
                s0 = h * P
                cur = half.tile([P, n_b], i32, tag="cur")
                nc.sync.dma_start(cur[:], text[s0 : s0 + P, :])
                prv = half.tile([P, n_b], i32, tag="prv")
                if h == 0:
                    nc.sync.dma_start(prv[0:1, :], text[0:1, :])
                    nc.sync.dma_start(prv[1:P, :], text[0 : P - 1, :])
                else:
                    nc.sync.dma_start(prv[:], text[s0 - 1 : s0 + P - 1, :])

                # flat trigram key = prev * 4096 + cur (exact, < 2^24)
                fk = half.tile([P, n_b], i32, tag="fk")
                nc.vector.scalar_tensor_tensor(
                    out=fk[:],
                    in0=prv[:],
                    scalar=V,
                    in1=cur[:],
                    op0=mybir.AluOpType.mult,
                    op1=mybir.AluOpType.add,
                )
                if h == 0 and k_miss is not None:
                    # seq positions 0,1 never take the trigram branch: remap
                    # their keys to a known tri_map miss
                    nc.vector.memset(fk[0:2, :], k_miss)
                curs.append(cur)
                fks.append(fk)

            # ---- phase 2: software-pipelined per-tile work ----
            # gpsimd (SWDGE Q7 descriptor emission + ring backpressure) and
            # HBM bytes are the scarce resources: keep the gpsimd queue free
            # of data-dependent stalls by running index prep LOOKAHEAD tiles
            # ahead, and keep the combine on the vector engine in cheap 2x
            # ops (tri tile zeroed, plain tensor add).
            tiles = [(b, h) for b in range(n_b) for h in range(n_h)]
            risks, hits, tris = {}, {}, {}

            def issue_prep(t):
                b, h = tiles[t]
                ridx = small.tile([P, 1], i32, tag="ridx")
                nc.gpsimd.indirect_dma_start(
                    out=ridx[:],
                    out_offset=None,
                    in_=tri_map[:],
                    in_offset=bass.IndirectOffsetOnAxis(
                        ap=fks[h][:, b : b + 1], axis=0
                    ),
                )
                if h == 0 and k_miss is None:
                    nc.vector.memset(ridx[0:2, :], -1)
                # miss (-1) -> 65535 which fails bounds_check -> skipped
                risk = small.tile([P, 1], i32, tag="risk")
                nc.vector.tensor_scalar(
                    out=risk[:],
                    in0=ridx[:],
                    scalar1=0xFFFF,
                    scalar2=None,
                    op0=mybir.AluOpType.bitwise_and,
                )
                # hit indicator in {0.0, 1.0}
                hit = small.tile([P, 1], f32, tag="hit")
                nc.vector.tensor_scalar(
                    out=hit[:],
                    in0=ridx[:],
                    scalar1=0,
                    scalar2=None,
                    op0=mybir.AluOpType.is_ge,
                )
                # zero the tri tile ahead of time so skipped (miss) rows
                # contribute exactly 0 to the add; u32 view halves the DVE
                # element count (same bytes)
                tri = tri_pool.tile([P, V], bf16, tag="tri")
                nc.vector.memset(tri[:].bitcast(mybir.dt.uint32), 0)
                risks[t] = risk
                hits[t] = hit
                tris[t] = tri

            bis = {}

            def issue_bi(t):
                b, h = tiles[t]
                bi = bi_pool.tile([P, VA], bf16, tag="bi")
                nc.gpsimd.indirect_dma_start(
                    out=bi[:],
                    out_offset=None,
                    in_=bigram[:],
                    in_offset=bass.IndirectOffsetOnAxis(
                        ap=curs[h][:, b : b + 1], axis=0
                    ),
                )
                bis[t] = bi

            def issue_tri(t):
                nc.gpsimd.indirect_dma_start(
                    out=tris[t][:],
                    out_offset=None,
                    in_=tri_rows[:],
                    in_offset=bass.IndirectOffsetOnAxis(
                        ap=risks[t][:, :1], axis=0
                    ),
                    bounds_check=K - 1,
                    oob_is_err=False,
                )

            def issue_compute(t):
                b, h = tiles[t]
                s0 = h * P
                bi, hit, tri = bis[t], hits[t], tris[t]

                # z = rowsum + 0.75*hit (sum col; EPS/ALPHA = 2.5e-10 is
                # below f32/bf16 resolution of z ~ 1.75 so reference's +EPS
                # in the denominator is a no-op here)
                z = small.tile([P, 1], f32, tag="z")
                nc.vector.scalar_tensor_tensor(
                    out=z[:],
                    in0=hit[:, :1],
                    scalar=R_TRI,
                    in1=bi[:, V : V + 1],
                    op0=mybir.AluOpType.mult,
                    op1=mybir.AluOpType.add,
                )
                r = small.tile([P, 1], f32, tag="r")
                nc.vector.reciprocal(r[:], z[:])

                # q = tri + bi (tri pre-scaled by 0.75, zero on miss; 2x TT)
                nc.vector.tensor_tensor(
                    out=bi[:, 0:V],
                    in0=tri[:],
                    in1=bi[:, 0:V],
                    op=mybir.AluOpType.add,
                )

                ot = out_pool.tile([P, V], bf16, tag="ot")
                nc.scalar.activation(
                    out=ot[:],
                    in_=bi[:, 0:V],
                    func=mybir.ActivationFunctionType.Ln,
                    bias=eps_b[:, :1],
                    scale=r[:, :1],
                )

                nc.sync.dma_start(out[s0 : s0 + P, b * V : (b + 1) * V], ot[:])

            # start the bulk read stream immediately (bigram gathers only
            # need the token ids), then index prep; trigram gathers trail
            # the bigram gathers by 2 tiles and compute trails by one more,
            # so no engine ever head-of-line-waits on a just-issued DMA
            for t in range(min(2, n_tiles)):
                issue_bi(t)
            for t in range(min(LOOKAHEAD, n_tiles)):
                issue_prep(t)
            for t in range(n_tiles + 1):
                # tri(t) first: its drain feeds compute(t) next iteration,
                # while bi(t+2) isn't consumed for two more
                if t < n_tiles:
                    issue_tri(t)
                if t + 2 < n_tiles:
                    issue_bi(t + 2)
                if t >= 1:
                    issue_compute(t - 1)
                if t + LOOKAHEAD < n_tiles:
                    issue_prep(t + LOOKAHEAD)

    nc.finalize()
    return nc


def _prep_inputs(text, unigram, bigram_table, tri_rows, tri_map):
    """Shared (replicated) device arrays, keyed by dram tensor name."""
    text = np.ascontiguousarray(np.asarray(text, dtype=np.int32))
    uni = np.asarray(unigram, np.float32).reshape(1, V)
    bt = np.asarray(bigram_table, np.float32) + R_UNI * uni  # fold unigram in
    aug = np.zeros((V, VA), dtype=BF16)
    aug[:, :V] = bt.astype(BF16)
    aug[:, V] = bt.sum(axis=1).astype(BF16)
    tr = np.ascontiguousarray(
        (np.asarray(tri_rows, np.float32) * R_TRI).astype(BF16)
    )
    tm = np.ascontiguousarray(np.asarray(tri_map, np.int32).reshape(V * V, 1))
    shared = {
        "text": text,
        "bigram_table": np.ascontiguousarray(aug),
        "tri_rows": np.ascontiguousarray(tr),
        "tri_map": tm,
    }
    # any key absent from the trigram dict (used to force seq pos 0,1 to miss)
    k_miss = int(np.flatnonzero(tm[:, 0] < 0)[0])
    return shared, k_miss


def make_in_maps(shared):
    text = shared["text"]
    in_maps = []
    for c in range(NCORES):
        m = dict(shared)
        m["text"] = np.ascontiguousarray(text[:, c * BS : (c + 1) * BS])
        in_maps.append(m)
    return in_maps


def kernel(text, unigram, bigram_table, tri_rows, tri_map, _trace=False, _trace_kwargs=None):
    shared, k_miss = _prep_inputs(text, unigram, bigram_table, tri_rows, tri_map)
    nc = build_nc(BS, k_miss=k_miss)
    in_maps = make_in_maps(shared)
    res = run_bass_kernel_spmd(
        nc,
        in_maps,
        core_ids=list(range(NCORES)),
        trace=_trace,
        **(_trace_kwargs or {}),
    )
    outs = [
        np.asarray(res.results[c]["out"]).astype(np.float32).reshape(S, BS, V)
        for c in range(NCORES)
    ]
    full = np.concatenate(outs, axis=1)
    if _trace:
        return full, res
    return full
